# revision 19
# baseline (speedup 1.0000x reference)
"""Trainium2 Bass kernel for the audio-visual attention model (v2: fp8 hybrid).

Math (per (b,t) sample, BT = 32*64 = 2048 of them):
    V   = video[b,t]                              # [48, 512]
    v   = relu(V @ W_video.T + b_video)           # [48, 512]
    a   = relu(audio[b,t] @ W_audio.T + b_audio)  # [512]
    inter   = a @ W_g.T                           # [48]
    content = v @ W_v.T + inter[:, None]          # [48, 48]
    z   = tanh(content) @ W_h.T                   # [48]
    alpha = softmax(z)
    out = alpha @ V                               # [512]

v2 changes vs the fp16 baseline (139us):
  * relu split:  relu(x) = (x + |x|)/2, so
        content = 0.5*|pre+b| @ W_v.T + 0.5*V @ (W_video.T @ W_v.T)
                  + 0.5*(b @ W_v.T) + inter
    The linear half rides an exact host-precomputed [512,48] Wcomb in fp16;
    only the |pre| half carries main-matmul error.
  * mains k-hybrid: contraction rows 0:256 run as ONE fp8e4 DoubleRow matmul
    (2 rows/cycle), rows 256:512 as two fp16 matmuls.  Host pre-scales
    V*16 / W*32 (fp8) and W*512 (fp16) so the PSUM holds 512*pre; the Abs
    activation rescales by 1/512.  Predicted rel err 1.47e-2 (sim), vs the
    2e-2 gate; pure-fp8 mains measure 2.9e-2 and are not usable.

Strategy: data-parallel over BT across 8 cores (256 samples each, R = 256*48
= 12288 video rows per core).  The host pre-transposes the video shard to
V.T [512, 12288] fp16 plus an fp8 copy of rows 0:256, pre-arranges weights
into device layouts, and runs the matmul chain per superblock as in v1:
vT->|pre| (PE+ACT), content.T via col-tiled concurrent chains (PE), tanh
(ACT), z row-tiled (PE), exp (ACT), weighted mul + halving-tree reduce
(DVE), outputs streamed per chunk on the gpsimd DMA ring.
"""

import numpy as np

# Problem constants (hardcoded per harness contract).
B, T = 32, 64
ASIZE, VSIZE, HSIZE, MSIZE = 128, 512, 512, 48
NCORES = 8
BT = B * T                     # 2048
PER = BT // NCORES             # 256 samples per core
R = PER * MSIZE                # 12288 video rows per core
SUPER = 3072                   # rows per superblock (64 groups of 48)
NSB = R // SUPER               # 4 superblocks
SUB = 512                      # matmul moving-dim block (PSUM bank limit)
DSUB = 2 * SUB                 # 1024-col double block for content/score
NPAIR = R // DSUB              # 12 double blocks
PPS = SUPER // DSUB            # 3 double blocks per superblock
GPS = SUPER // MSIZE           # 64 sample groups per superblock
FGRP = 16                      # groups per finalize chunk

K8 = 256                       # contraction rows 0:K8 in fp8 DoubleRow
V8_SCALE = 16.0
W8_SCALE = 32.0
PRE_SCALE = V8_SCALE * W8_SCALE  # PSUM holds PRE_SCALE * pre

_cached = {}


def _build_nc():
    import concourse.bacc as bacc
    import concourse.mybir as mybir
    import concourse.tile as tile

    f32 = mybir.dt.float32
    f16 = mybir.dt.float16
    f8 = mybir.dt.float8e4
    AF = mybir.ActivationFunctionType
    AX = mybir.AxisListType
    DR = mybir.MatmulPerfMode.DoubleRow

    nc = bacc.Bacc(
        "TRN2",
        target_bir_lowering=False,
        debug=False,
        enable_asserts=False,
        num_devices=NCORES,
    )

    # ---- DRAM I/O ----
    vT_d = nc.dram_tensor("vT", [VSIZE, R], f16, kind="ExternalInput").ap()
    vT8_d = nc.dram_tensor("vT8", [K8, R], f8, kind="ExternalInput").ap()
    audioT_d = nc.dram_tensor("audioT", [ASIZE, PER], f16, kind="ExternalInput").ap()
    wvideoT8_d = nc.dram_tensor("WvideoT8", [128, K8 // 128, HSIZE], f8, kind="ExternalInput").ap()
    w16h_d = nc.dram_tensor("W16h", [128, (VSIZE - K8) // 128, HSIZE], f16, kind="ExternalInput").ap()
    waudioT_d = nc.dram_tensor("WaudioT", [ASIZE, HSIZE], f16, kind="ExternalInput").ap()
    wgT_d = nc.dram_tensor("WgT", [128, HSIZE // 128, MSIZE], f16, kind="ExternalInput").ap()
    wvT_d = nc.dram_tensor("WvT", [128, HSIZE // 128, MSIZE], f16, kind="ExternalInput").ap()
    wcomb_d = nc.dram_tensor("Wcomb", [128, VSIZE // 128, MSIZE], f16, kind="ExternalInput").ap()
    whT_d = nc.dram_tensor("WhT", [112, 1], f32, kind="ExternalInput").ap()
    bvideo_d = nc.dram_tensor("b_video", [128, HSIZE // 128], f32, kind="ExternalInput").ap()
    baudio_d = nc.dram_tensor("b_audio", [128, HSIZE // 128], f32, kind="ExternalInput").ap()
    cbias_d = nc.dram_tensor("cbias", [112, 1], f32, kind="ExternalInput").ap()
    cT_d = nc.dram_tensor("cT", [VSIZE, PER], f16, kind="ExternalOutput").ap()
    # ez row 0 per sample-column; the host computes denom = group-sums of 48
    # in fp32 (cheaper and more accurate than on-device fp16 reduces)
    ezrow_d = nc.dram_tensor("ezrow", [1, R], f16, kind="ExternalOutput").ap()

    KC = VSIZE // 128          # 4 v chunks (weighted mul / Wcomb contraction)
    K16C = (VSIZE - K8) // 128  # 2 fp16 mains chunks
    HC = HSIZE // 128          # 4 h chunks

    with tile.TileContext(nc) as tc:
        with (
            tc.tile_pool(name="const", bufs=1) as const,
        ):
            # ---- constants / weights.  Audio-path tensors go on the scalar
            # ring (they gate the first PE work); the big main-loop weights go
            # on the gpsimd ring, wvideoT8/w16h first -- they gate mains. ----
            audioT_sb = const.tile([128, PER], f16)
            waudioT_sb = const.tile([128, HSIZE], f16)
            baudio_sb = const.tile([128, HC], f32)
            wgT_sb = const.tile([128, HC, MSIZE], f16)
            wvideoT8_sb = const.tile([128, K8 // 128, HSIZE], f8)
            nc.gpsimd.dma_start(out=wvideoT8_sb, in_=wvideoT8_d)
            w16h_sb = const.tile([128, K16C, HSIZE], f16)
            nc.gpsimd.dma_start(out=w16h_sb, in_=w16h_d)
            bvideo_sb = const.tile([128, HC], f32)
            nc.gpsimd.dma_start(out=bvideo_sb, in_=bvideo_d)
            wvT_sb = const.tile([128, HC, MSIZE], f16)
            nc.gpsimd.dma_start(out=wvT_sb, in_=wvT_d)
            wcomb_sb = const.tile([128, KC, MSIZE], f16)
            nc.gpsimd.dma_start(out=wcomb_sb, in_=wcomb_d)
            whT_sb = const.tile([112, 1], f32)
            nc.gpsimd.dma_start(out=whT_sb, in_=whT_d)
            cbias_sb = const.tile([112, 1], f32)
            nc.gpsimd.dma_start(out=cbias_sb, in_=cbias_d)
            ones_m = const.tile([112, 128], f32)
            nc.vector.memset(ones_m, 1.0)
            # W_h replicated across 128 free cols, on partitions 0-47 AND
            # 64-111 (rows 48-63 zero) for the two row-tiled z matmuls
            whB_sb = const.tile([112, 128], f16)
            nc.scalar.mul(out=whB_sb, in_=ones_m, mul=whT_sb)
            # HAM warm-up: keep the PE busy during the initial DMA fill so the
            # clock gate is at 8/8 (2.4 GHz) before the real matmuls arrive
            warm_sb = const.tile([128, 64], f16)
            nc.vector.memset(warm_sb.bitcast(f32), 0.0)
            ones_f32 = const.tile([1, 128], f32)
            nc.vector.memset(ones_f32, 1.0)
            ones48 = const.tile([1, MSIZE], f16)
            nc.vector.tensor_copy(out=ones48, in_=ones_f32[:, :MSIZE])

            # persistent accumulators
            cT_acc = const.tile([128, KC, PER], f16)
            interflat_all = const.tile([1, R], f16)

            with (
                tc.tile_pool(name="vt", bufs=3) as vtp,
                tc.tile_pool(name="vt8", bufs=3) as vt8p,
                tc.tile_pool(name="vrelu", bufs=2) as vrp,
                tc.tile_pool(name="tanhp", bufs=2) as thp,
                tc.tile_pool(name="ezb", bufs=2) as ezp,
                tc.tile_pool(name="tree", bufs=2) as trp,
                tc.tile_pool(name="mm_ps", bufs=2, space="PSUM") as mm_ps,
                tc.tile_pool(name="ct_ps", bufs=1, space="PSUM") as ct_ps,
                tc.tile_pool(name="z_ps", bufs=1, space="PSUM") as z_ps,
            ):
                vt_t, vt8_t, vr_t, th_t, ez_t = {}, {}, {}, {}, {}

                def alloc_sb(sb):
                    # allocate the superblock's tiles and issue their DMAs.
                    # sb 0 is chunked (small first transfers so the first
                    # matmul starts early, fp8 first since it heads the PSUM
                    # chain); later sbs stream whole on the sync ring.
                    vt_t[sb] = vtp.tile([128, KC, SUPER], f16, tag="vt",
                                        name=f"vt_{sb}")
                    vt8_t[sb] = vt8p.tile([128, K8 // 128, SUPER], f8,
                                          tag="vt8", name=f"vt8_{sb}")
                    if sb == 0:
                        for i in range(K8 // 128):
                            nc.sync.dma_start(
                                out=vt8_t[sb][:, i, 0:DSUB],
                                in_=vT8_d[i * 128 : (i + 1) * 128, 0:DSUB],
                            )
                        # the two fp16 chunks the first mains chain needs go
                        # on the scalar ring (ahead of the audio weights) so
                        # they issue in parallel with the sync-ring fp8 chunks
                        for k in range(K8 // 128, KC):
                            nc.scalar.dma_start(
                                out=vt_t[sb][:, k, 0:DSUB],
                                in_=vT_d[k * 128 : (k + 1) * 128, 0:DSUB],
                            )
                        # strict need-order on the sync ring: pair-1 data
                        # (cc=1) before the k0/k1 first chunks (only needed
                        # by content(0), two pairs later), then cc=2.  This
                        # keeps the first mains' critical transfers (vt8 +
                        # scalar-ring k2/k3 + gpsimd-ring weights) from
                        # queuing behind ~2MB of bulk.
                        nc.sync.dma_start(
                            out=vt8_t[sb][:, :, DSUB : 2 * DSUB],
                            in_=vT8_d[:, DSUB : 2 * DSUB].rearrange(
                                "(c p) n -> p c n", p=128
                            ),
                        )
                        nc.sync.dma_start(
                            out=vt_t[sb][:, :, DSUB : 2 * DSUB],
                            in_=vT_d[:, DSUB : 2 * DSUB].rearrange(
                                "(c p) n -> p c n", p=128
                            ),
                        )
                        for k in range(K8 // 128):
                            nc.sync.dma_start(
                                out=vt_t[sb][:, k, 0:DSUB],
                                in_=vT_d[k * 128 : (k + 1) * 128, 0:DSUB],
                            )
                        nc.sync.dma_start(
                            out=vt8_t[sb][:, :, 2 * DSUB : PPS * DSUB],
                            in_=vT8_d[:, 2 * DSUB : PPS * DSUB].rearrange(
                                "(c p) n -> p c n", p=128
                            ),
                        )
                        nc.sync.dma_start(
                            out=vt_t[sb][:, :, 2 * DSUB : PPS * DSUB],
                            in_=vT_d[:, 2 * DSUB : PPS * DSUB].rearrange(
                                "(c p) n -> p c n", p=128
                            ),
                        )
                    else:
                        nc.sync.dma_start(
                            out=vt8_t[sb],
                            in_=vT8_d[:, sb * SUPER : (sb + 1) * SUPER].rearrange(
                                "(c p) n -> p c n", p=128
                            ),
                        )
                        nc.sync.dma_start(
                            out=vt_t[sb],
                            in_=vT_d[:, sb * SUPER : (sb + 1) * SUPER].rearrange(
                                "(c p) n -> p c n", p=128
                            ),
                        )
                    vr_t[sb] = vrp.tile([128, HC, SUPER], f16, tag="vrelu",
                                        name=f"vrelu_{sb}")
                    # tanh halves: col half A on partitions 0-47, half B on
                    # 64-111, both at free offset 512p (same ACT op)
                    th_t[sb] = thp.tile([112, SUPER // 2], f16, tag="tanhc",
                                        name=f"tanhc_{sb}")
                    ez_t[sb] = ezp.tile([128, SUPER], f16, tag="ezb",
                                        name=f"ezb_{sb}")

                # issue the first superblock's DMAs BEFORE the warm burst so
                # the sync/scalar sequencers start moving data immediately;
                # the warm matmuls then cover exactly the remaining fill time.
                # The audio-path loads follow on the scalar ring (needed a
                # few us later than the first mains chunks).
                alloc_sb(0)
                nc.scalar.dma_start(out=audioT_sb, in_=audioT_d)
                nc.scalar.dma_start(out=waudioT_sb, in_=waudioT_d)
                nc.scalar.dma_start(out=baudio_sb, in_=baudio_d)
                nc.scalar.dma_start(out=wgT_sb, in_=wgT_d)

                warm_ps = mm_ps.tile([64, 64], f32, tag="v_ps", name="warm_ps")

                def warm_burst(n):
                    for _ in range(n):
                        nc.tensor.matmul(
                            warm_ps, warm_sb[:, :64], warm_sb, start=True, stop=True
                        )

                warm_burst(58)

                aT_sb = const.tile([128, HC, PER], f16)

                def emit_audio_a():
                    # a.T = relu(W_audio.T^T @ audio.T + b_audio); emitted
                    # between the first two mains half-pairs so its ACT relus
                    # queue behind only two abs ops instead of four
                    for m in range(HC):
                        a_ps = mm_ps.tile([128, PER], f32, tag="v_ps",
                                          name=f"a_ps_{m}")
                        nc.tensor.matmul(
                            a_ps,
                            waudioT_sb[:, m * 128 : (m + 1) * 128],
                            audioT_sb,
                            start=True,
                            stop=True,
                        )
                        nc.scalar.activation(
                            out=aT_sb[:, m, :], in_=a_ps, func=AF.Relu,
                            bias=baudio_sb[:, m : m + 1],
                        )

                def emit_audio_inter():
                    # inter[bt, m] = a @ W_g.T, natural layout for a flat write
                    inter_sb = const.tile([128, PER // 128, MSIZE], f16)
                    for t in range(PER // 128):
                        i_ps = mm_ps.tile([128, MSIZE], f32, tag="v_ps",
                                          name=f"i_ps_{t}")
                        for k in range(HC):
                            nc.tensor.matmul(
                                i_ps,
                                aT_sb[:, k, t * 128 : (t + 1) * 128],
                                wgT_sb[:, k, :],
                                start=(k == 0),
                                stop=(k == HC - 1),
                            )
                        nc.scalar.copy(out=inter_sb[:, t, :], in_=i_ps)
                    # flatten inter [bt, m] row-major into a single-partition
                    # row via SBUF->SBUF DMA (no HBM roundtrip)
                    for t in range(PER // 128):
                        nc.gpsimd.dma_start(
                            out=interflat_all[
                                :, t * 128 * MSIZE : (t + 1) * 128 * MSIZE
                            ],
                            in_=inter_sb[:, t, :],
                        )

                def emit_mains(q, ms):
                    sb, p = divmod(q, PPS)
                    # prefetch the next superblock's tiles one pair earlier
                    # than first use (bufs=3 pools absorb the extra lifetime)
                    if p == 1 and ms[0] == 0 and sb + 1 < NSB:
                        alloc_sb(sb + 1)
                    c0 = p * DSUB
                    for m in ms:
                        # both 512-col halves of this m-chunk accumulate into
                        # one 2-bank PSUM tile -> ONE fused |pre+b| of 1024
                        # cols.  Chain per half: 1 fp8 DoubleRow MM (rows
                        # 0:256, 2 rows/cycle) + 2 fp16 MMs (rows 256:512).
                        # weight-major order: each of the 3 stationary weights
                        # (fp16 a, fp16 b, fp8 pair) is loaded ONCE and runs
                        # both 512-col halves back-to-back.  The fp16 chunks
                        # go FIRST and the fp8 DoubleRow chunk LAST, so the
                        # 256-col DR weight load (213ns, no pull-ahead) hides
                        # under the two preceding 213ns fp16 matmuls, and the
                        # next m's 128-col fp16 load hides under the DR pair.
                        # On pair 0 the DR chunk goes FIRST instead: its only
                        # inputs (vt8 + the fp8 weight, 384KB) land ~3us
                        # before the fp16 weight/chunks, so the PE starts on
                        # DR work while the rest of the critical set streams.
                        v_ps = mm_ps.tile([128, 2, SUB], f32, tag="v_ps",
                                          name=f"v_ps_{q}_{m}")
                        dr_first = q == 0
                        if dr_first:
                            for j in range(2):
                                nc.tensor.matmul(
                                    v_ps[:, j, :],
                                    wvideoT8_sb[:, :, m * 128 : (m + 1) * 128],
                                    vt8_t[sb][:, :, c0 + j * SUB : c0 + (j + 1) * SUB],
                                    start=True,
                                    stop=False,
                                    perf_mode=DR,
                                )
                        for c in range(K16C):
                            for j in range(2):
                                nc.tensor.matmul(
                                    v_ps[:, j, :],
                                    w16h_sb[:, c, m * 128 : (m + 1) * 128],
                                    vt_t[sb][:, K8 // 128 + c,
                                             c0 + j * SUB : c0 + (j + 1) * SUB],
                                    start=(c == 0 and not dr_first),
                                    stop=(dr_first and c == K16C - 1),
                                )
                        if not dr_first:
                            for j in range(2):
                                nc.tensor.matmul(
                                    v_ps[:, j, :],
                                    wvideoT8_sb[:, :, m * 128 : (m + 1) * 128],
                                    vt8_t[sb][:, :, c0 + j * SUB : c0 + (j + 1) * SUB],
                                    start=False,
                                    stop=True,
                                    perf_mode=DR,
                                )
                        nc.scalar.activation(
                            out=vr_t[sb][:, m, c0 : c0 + DSUB].rearrange(
                                "p (a b) -> p a b", a=2
                            ),
                            in_=v_ps, func=AF.Abs,
                            bias=bvideo_sb[:, m : m + 1],
                            scale=1.0 / PRE_SCALE,
                        )

                def emit_content(q):
                    # two 512-col half chains packed onto array col groups
                    # 0-47 / 64-111; the chains run concurrently on the PE.
                    # chain: inter broadcast + 4x (0.5*W_v.T : |pre+b|)
                    #        + 4x (Wcomb : vT)   [Wcomb = 0.5*W_video.T@W_v.T]
                    sb, p = divmod(q, PPS)
                    c0 = p * DSUB
                    r0 = sb * SUPER
                    ct = ct_ps.tile([128, SUB], f32, tag="ct", name=f"ct_{q}")
                    ctA = ct[0:MSIZE, :]
                    ctB = ct[64 : 64 + MSIZE, :]
                    nc.tensor.matmul(
                        ctA, ones48,
                        interflat_all[:, r0 + c0 : r0 + c0 + SUB],
                        start=True, stop=False,
                    )
                    nc.tensor.matmul(
                        ctB, ones48,
                        interflat_all[:, r0 + c0 + SUB : r0 + c0 + DSUB],
                        start=True, stop=False,
                    )
                    for k in range(HC):
                        nc.tensor.matmul(
                            ctA, wvT_sb[:, k, :], vr_t[sb][:, k, c0 : c0 + SUB],
                            start=False, stop=False,
                        )
                        nc.tensor.matmul(
                            ctB, wvT_sb[:, k, :],
                            vr_t[sb][:, k, c0 + SUB : c0 + DSUB],
                            start=False, stop=False,
                        )
                    for k in range(KC):
                        nc.tensor.matmul(
                            ctA, wcomb_sb[:, k, :], vt_t[sb][:, k, c0 : c0 + SUB],
                            start=False, stop=(k == KC - 1),
                        )
                        nc.tensor.matmul(
                            ctB, wcomb_sb[:, k, :],
                            vt_t[sb][:, k, c0 + SUB : c0 + DSUB],
                            start=False, stop=(k == KC - 1),
                        )
                    # one fused tanh over partitions 0-111 (rows 48-63 junk);
                    # cbias = 0.5*(b_video @ W_v.T) rides the ACT bias
                    nc.scalar.activation(
                        out=th_t[sb][0:112, p * SUB : (p + 1) * SUB],
                        in_=ct[0:112, :], func=AF.Tanh,
                        bias=cbias_sb,
                    )

                def emit_zmm(q):
                    sb, p = divmod(q, PPS)
                    # two row-tiled z matmuls (array rows 0-47 / 64-111) into
                    # the two banks of one PSUM tile, concurrent on the PE
                    zt = z_ps.tile([128, 2, SUB], f32, tag="zt", name=f"z_{q}")
                    nc.tensor.matmul(
                        zt[:, 0, :], whB_sb[0:MSIZE, :],
                        th_t[sb][0:MSIZE, p * SUB : (p + 1) * SUB],
                        start=True, stop=True,
                    )
                    nc.tensor.matmul(
                        zt[:, 1, :], whB_sb[64 : 64 + MSIZE, :],
                        th_t[sb][64 : 64 + MSIZE, p * SUB : (p + 1) * SUB],
                        start=True, stop=True,
                    )
                    return zt

                def emit_expmul(q, zt, j=None):
                    # exp + in-place weighted multiply; j=None does the full
                    # DSUB in one fused exp + one mul, j=0/1 does one half
                    sb, p = divmod(q, PPS)
                    c0 = p * DSUB
                    if j is None:
                        nc.scalar.activation(
                            out=ez_t[sb][:, c0 : c0 + DSUB].rearrange(
                                "p (a b) -> p a b", a=2
                            ),
                            in_=zt, func=AF.Exp,
                        )
                        nc.vector.tensor_mul(
                            vt_t[sb][:, :, c0 : c0 + DSUB],
                            vt_t[sb][:, :, c0 : c0 + DSUB],
                            ez_t[sb][:, c0 : c0 + DSUB]
                            .unsqueeze(1)
                            .broadcast_to([128, KC, DSUB]),
                        )
                    else:
                        cj = c0 + j * SUB
                        nc.scalar.activation(
                            out=ez_t[sb][:, cj : cj + SUB], in_=zt[:, j, :],
                            func=AF.Exp,
                        )
                        nc.vector.tensor_mul(
                            vt_t[sb][:, :, cj : cj + SUB],
                            vt_t[sb][:, :, cj : cj + SUB],
                            ez_t[sb][:, cj : cj + SUB]
                            .unsqueeze(1)
                            .broadcast_to([128, KC, SUB]),
                        )

                def emit_score(q):
                    emit_expmul(q, emit_zmm(q))

                def emit_finalize(sb, g0f, ng, dma=None, ring=None):
                    # reduce cols [g0f*48, (g0f+ng)*48) of superblock sb
                    # (units of 48-col sample groups, local to sb).
                    # dma=(gd0, ngd) flushes that global group range of
                    # cT_acc to DRAM.
                    ch = g0f * MSIZE
                    g0 = sb * GPS + g0f
                    lp = nc.allow_low_precision(
                        reason="fp16 group sums; fp32 internal accum"
                    )
                    lp.__enter__()
                    tree = trp.tile([128, KC, FGRP, MSIZE // 2], f16, tag="tree",
                                    name=f"tree_{sb}_{g0f}")
                    wv = vt_t[sb][:, :, ch : ch + ng * MSIZE].rearrange(
                        "p c (g n) -> p c g n", n=MSIZE
                    )
                    tr = tree[:, :, :ng, :]
                    nc.vector.tensor_add(
                        tr, wv[:, :, :, : MSIZE // 2], wv[:, :, :, MSIZE // 2 :]
                    )
                    nc.vector.tensor_add(
                        tr[:, :, :, : MSIZE // 4],
                        tr[:, :, :, : MSIZE // 4],
                        tr[:, :, :, MSIZE // 4 :],
                    )
                    nc.vector.tensor_add(
                        tr[:, :, :, : MSIZE // 8],
                        tr[:, :, :, : MSIZE // 8],
                        tr[:, :, :, MSIZE // 8 : MSIZE // 4],
                    )
                    nc.vector.reduce_sum(
                        out=cT_acc[:, :, g0 : g0 + ng],
                        in_=tr[:, :, :, : MSIZE // 8],
                        axis=AX.X,
                    )
                    lp.__exit__(None, None, None)
                    if dma is not None:
                        gd0, ngd = dma
                        (ring or nc.gpsimd).dma_start(
                            out=cT_d[:, gd0 : gd0 + ngd].rearrange(
                                "(c p) n -> p c n", p=128
                            ),
                            in_=cT_acc[:, :, gd0 : gd0 + ngd],
                        )

                def emit_ezrow(sb, ring=None):
                    (ring or nc.gpsimd).dma_start(
                        out=ezrow_d[:, sb * SUPER : (sb + 1) * SUPER],
                        in_=ez_t[sb][0:1, :],
                    )

                # software-pipelined emission with a ONE-pair lag, interleaved
                # at half-pair granularity so no consumer head-of-line-blocks
                # the PE queue (see v1 docstring for the full rationale).
                lsb = NSB - 1

                def emit_fins(qq):
                    sb2, p2 = divmod(qq, PPS)
                    if p2 == 0:
                        emit_finalize(sb2, 0, FGRP)
                    elif p2 == 1:
                        emit_finalize(sb2, FGRP, FGRP)
                        emit_finalize(sb2, 2 * FGRP, 10)
                    else:
                        emit_finalize(sb2, 42, 11)
                        emit_finalize(sb2, 53, 11, dma=(sb2 * GPS, GPS))
                        emit_ezrow(sb2)

                def emit_consume(qq):
                    sb2, p2 = divmod(qq, PPS)
                    if sb2 < lsb:
                        emit_score(qq)
                        emit_fins(qq)
                        return
                    # last superblock: exp/mul per 512-col half, with the
                    # largest group-aligned chunk each half unlocks
                    zt = emit_zmm(qq)
                    emit_expmul(qq, zt, j=0)
                    if p2 == 0:
                        emit_finalize(lsb, 0, 10)
                    elif p2 == 1:
                        emit_finalize(lsb, 21, 11)
                    else:
                        emit_finalize(lsb, 42, 11, dma=(lsb * GPS + 42, 11))
                    emit_expmul(qq, zt, j=1)
                    if p2 == 0:
                        emit_finalize(lsb, 10, 11)
                    elif p2 == 1:
                        emit_finalize(lsb, 32, 10, dma=(lsb * GPS, 42))
                    else:
                        # ezrow first (only needs the exps, long done); the
                        # final finalize is split so the very last cT DMA is
                        # small and issues ~1us earlier.  The three tail DMAs
                        # issue on three different rings (the ~0.9us hwdge
                        # descriptor-gen cost overlaps instead of serializing)
                        emit_ezrow(lsb, ring=nc.scalar)
                        emit_finalize(lsb, 53, 6, dma=(lsb * GPS + 53, 6),
                                      ring=nc.sync)
                        emit_finalize(lsb, 59, 5, dma=(lsb * GPS + 59, 5),
                                      ring=nc.scalar)

                emit_mains(0, (0, 1))
                emit_audio_a()
                emit_mains(0, (2, 3))
                emit_audio_inter()
                for q in range(1, NPAIR):
                    emit_mains(q, (0, 1))
                    emit_content(q - 1)
                    # consume between m2 and m3: content(q-1)'s tanh drains
                    # during m2, so the z matmuls slot in without stalling
                    # and the DVE work starts ~1.7us earlier
                    emit_mains(q, (2,))
                    emit_consume(q - 1)
                    emit_mains(q, (3,))
                emit_content(NPAIR - 1)
                emit_consume(NPAIR - 1)

    nc.compile()
    return nc


def _prep_in_maps(inputs):
    import ml_dtypes

    E4M3 = ml_dtypes.float8_e4m3

    audio = np.ascontiguousarray(np.asarray(inputs["audio"], np.float32))
    video = np.ascontiguousarray(np.asarray(inputs["video"], np.float32))

    def dev_chunks(w):  # [C*128, X] -> [128, C, X] (partition-major chunks)
        a = np.asarray(w)
        return np.ascontiguousarray(a.reshape(-1, 128, a.shape[-1]).transpose(1, 0, 2))

    W_video = np.asarray(inputs["W_video"], np.float32)   # [h, v]
    W_videoT = W_video.T                                  # [v, h]
    # fp8 DoubleRow half: rows 0:K8, pre-scaled by W8_SCALE
    WvideoT8 = np.ascontiguousarray(
        (W_videoT[:K8] * W8_SCALE).astype(E4M3)
        .reshape(K8 // 128, 128, HSIZE).transpose(1, 0, 2)
    )
    # fp16 half: rows K8:, pre-scaled by PRE_SCALE (power of 2, exact)
    W16h = np.ascontiguousarray(
        (W_videoT[K8:] * PRE_SCALE).astype(np.float16)
        .reshape(-1, 128, HSIZE).transpose(1, 0, 2)
    )
    WaudioT = np.ascontiguousarray(np.asarray(inputs["W_audio"], np.float32).T.astype(np.float16))
    WgT = dev_chunks(np.asarray(inputs["W_g"], np.float32).T.astype(np.float16))
    W_v = np.asarray(inputs["W_v"], np.float32)
    WvT = dev_chunks((0.5 * W_v.T).astype(np.float16))
    # Wcomb = 0.5 * W_video.T @ W_v.T, computed exactly in float64 on host
    Wcomb = dev_chunks(
        (0.5 * (W_videoT.astype(np.float64) @ W_v.T.astype(np.float64)))
        .astype(np.float16)
    )
    wh = np.asarray(inputs["W_h"], np.float32).T  # [48, 1]
    WhT = np.zeros((112, 1), np.float32)
    WhT[0:MSIZE] = wh
    WhT[64 : 64 + MSIZE] = wh
    WhT = np.ascontiguousarray(WhT)
    b_video = np.asarray(inputs["b_video"], np.float32)
    b_videoT = np.ascontiguousarray(b_video.reshape(-1, 128).T)
    b_audio = np.ascontiguousarray(
        np.asarray(inputs["b_audio"], np.float32).reshape(-1, 128).T
    )
    # cbias = 0.5 * (b_video @ W_v.T), on partitions 0-47 and 64-111
    cb = 0.5 * (b_video @ W_v.T)
    cbias = np.zeros((112, 1), np.float32)
    cbias[0:MSIZE, 0] = cb
    cbias[64 : 64 + MSIZE, 0] = cb
    cbias = np.ascontiguousarray(cbias)

    a2 = audio.reshape(BT, ASIZE).astype(np.float16)
    v2f = video.reshape(BT, MSIZE, VSIZE)
    v2h = v2f.astype(np.float16)
    in_maps = []
    for c in range(NCORES):
        sl = slice(c * PER, (c + 1) * PER)
        vT = np.ascontiguousarray(v2h[sl].reshape(R, VSIZE).T)
        # fp8 copy of contraction rows 0:K8, scaled by V8_SCALE, quantized
        # from the fp32 source (matches the accuracy sim)
        vT8 = np.ascontiguousarray(
            (v2f[sl].reshape(R, VSIZE).T[:K8] * V8_SCALE).astype(E4M3)
        )
        audioT = np.ascontiguousarray(a2[sl].T)
        in_maps.append(
            {
                "vT": vT,
                "vT8": vT8,
                "audioT": audioT,
                "WvideoT8": WvideoT8,
                "W16h": W16h,
                "WaudioT": WaudioT,
                "WgT": WgT,
                "WvT": WvT,
                "Wcomb": Wcomb,
                "WhT": WhT,
                "b_video": b_videoT,
                "b_audio": b_audio,
                "cbias": cbias,
            }
        )
    return in_maps


def _run(inputs, trace=False, **spmd_kwargs):
    from concourse.bass_utils import run_bass_kernel_spmd

    if "nc" not in _cached:
        _cached["nc"] = _build_nc()
    nc = _cached["nc"]
    in_maps = _prep_in_maps(inputs)
    res = run_bass_kernel_spmd(
        nc, in_maps, core_ids=list(range(NCORES)), trace=trace, **spmd_kwargs
    )
    def _part(r):
        denom = r["ezrow"].astype(np.float32).reshape(PER, MSIZE).sum(axis=1)
        return (r["cT"].astype(np.float32) / denom[None, :]).T

    parts = [_part(r) for r in res.results]
    out = np.concatenate(parts, axis=0).reshape(B, T, VSIZE)
    return np.ascontiguousarray(out.astype(np.float32)), res


def kernel(**inputs):
    out, _ = _run(inputs, trace=False)
    return out


# revision 20
# speedup vs baseline: 1.1912x; 1.1912x over previous
"""Trainium2 Bass kernel for the audio-visual attention model (v2: fp8 hybrid).

Math (per (b,t) sample, BT = 32*64 = 2048 of them):
    V   = video[b,t]                              # [48, 512]
    v   = relu(V @ W_video.T + b_video)           # [48, 512]
    a   = relu(audio[b,t] @ W_audio.T + b_audio)  # [512]
    inter   = a @ W_g.T                           # [48]
    content = v @ W_v.T + inter[:, None]          # [48, 48]
    z   = tanh(content) @ W_h.T                   # [48]
    alpha = softmax(z)
    out = alpha @ V                               # [512]

v2 changes vs the fp16 baseline (139us):
  * relu split:  relu(x) = (x + |x|)/2, so
        content = 0.5*|pre+b| @ W_v.T + 0.5*V @ (W_video.T @ W_v.T)
                  + 0.5*(b @ W_v.T) + inter
    The linear half rides an exact host-precomputed [512,48] Wcomb in fp16;
    only the |pre| half carries main-matmul error.
  * mains k-hybrid: contraction rows 0:256 run as ONE fp8e4 DoubleRow matmul
    (2 rows/cycle), rows 256:512 as two fp16 matmuls.  Host pre-scales
    V*16 / W*32 (fp8) and W*512 (fp16) so the PSUM holds 512*pre; the Abs
    activation rescales by 1/512.  Predicted rel err 1.47e-2 (sim), vs the
    2e-2 gate; pure-fp8 mains measure 2.9e-2 and are not usable.

Strategy: data-parallel over BT across 8 cores (256 samples each, R = 256*48
= 12288 video rows per core).  The host pre-transposes the video shard to
V.T [512, 12288] fp16 plus an fp8 copy of rows 0:256, pre-arranges weights
into device layouts, and runs the matmul chain per superblock as in v1:
vT->|pre| (PE+ACT), content.T via col-tiled concurrent chains (PE), tanh
(ACT), z row-tiled (PE), exp (ACT), weighted mul + halving-tree reduce
(DVE), outputs streamed per chunk on the gpsimd DMA ring.
"""

import numpy as np

# Problem constants (hardcoded per harness contract).
B, T = 32, 64
ASIZE, VSIZE, HSIZE, MSIZE = 128, 512, 512, 48
NCORES = 8
BT = B * T                     # 2048
PER = BT // NCORES             # 256 samples per core
R = PER * MSIZE                # 12288 video rows per core
SUPER = 3072                   # rows per superblock (64 groups of 48)
NSB = R // SUPER               # 4 superblocks
SUB = 512                      # matmul moving-dim block (PSUM bank limit)
DSUB = 2 * SUB                 # 1024-col double block for content/score
NPAIR = R // DSUB              # 12 double blocks
PPS = SUPER // DSUB            # 3 double blocks per superblock
GPS = SUPER // MSIZE           # 64 sample groups per superblock
FGRP = 16                      # groups per finalize chunk

K8 = 256                       # contraction rows 0:K8 in fp8 DoubleRow
V8_SCALE = 16.0
W8_SCALE = 32.0
PRE_SCALE = V8_SCALE * W8_SCALE  # PSUM holds PRE_SCALE * pre

_cached = {}


def _build_nc():
    import concourse.bacc as bacc
    import concourse.mybir as mybir
    import concourse.tile as tile

    f32 = mybir.dt.float32
    f16 = mybir.dt.float16
    f8 = mybir.dt.float8e4
    AF = mybir.ActivationFunctionType
    AX = mybir.AxisListType
    DR = mybir.MatmulPerfMode.DoubleRow

    nc = bacc.Bacc(
        "TRN2",
        target_bir_lowering=False,
        debug=False,
        enable_asserts=False,
        num_devices=NCORES,
    )

    # ---- DRAM I/O ----
    vT_d = nc.dram_tensor("vT", [VSIZE, R], f16, kind="ExternalInput").ap()
    vT8_d = nc.dram_tensor("vT8", [K8, R], f8, kind="ExternalInput").ap()
    audioT_d = nc.dram_tensor("audioT", [ASIZE, PER], f16, kind="ExternalInput").ap()
    wvideoT8_d = nc.dram_tensor("WvideoT8", [128, K8 // 128, HSIZE], f8, kind="ExternalInput").ap()
    w16h_d = nc.dram_tensor("W16h", [128, (VSIZE - K8) // 128, HSIZE], f16, kind="ExternalInput").ap()
    waudioT_d = nc.dram_tensor("WaudioT", [ASIZE, HSIZE], f16, kind="ExternalInput").ap()
    wgT_d = nc.dram_tensor("WgT", [128, HSIZE // 128, MSIZE], f16, kind="ExternalInput").ap()
    wvT_d = nc.dram_tensor("WvT", [128, HSIZE // 128, MSIZE], f16, kind="ExternalInput").ap()
    wcomb_d = nc.dram_tensor("Wcomb", [128, VSIZE // 128, MSIZE], f16, kind="ExternalInput").ap()
    whT_d = nc.dram_tensor("WhT", [112, 1], f32, kind="ExternalInput").ap()
    bvideo_d = nc.dram_tensor("b_video", [128, HSIZE // 128], f32, kind="ExternalInput").ap()
    baudio_d = nc.dram_tensor("b_audio", [128, HSIZE // 128], f32, kind="ExternalInput").ap()
    cbias_d = nc.dram_tensor("cbias", [112, 1], f32, kind="ExternalInput").ap()
    cT_d = nc.dram_tensor("cT", [VSIZE, PER], f16, kind="ExternalOutput").ap()
    # ez row 0 per sample-column; the host computes denom = group-sums of 48
    # in fp32 (cheaper and more accurate than on-device fp16 reduces)
    ezrow_d = nc.dram_tensor("ezrow", [1, R], f16, kind="ExternalOutput").ap()

    KC = VSIZE // 128          # 4 v chunks (weighted mul / Wcomb contraction)
    K16C = (VSIZE - K8) // 128  # 2 fp16 mains chunks
    HC = HSIZE // 128          # 4 h chunks

    with tile.TileContext(nc) as tc:
        with (
            tc.tile_pool(name="const", bufs=1) as const,
        ):
            # ---- constants / weights.  Audio-path tensors go on the scalar
            # ring (they gate the first PE work); the big main-loop weights go
            # on the gpsimd ring, wvideoT8/w16h first -- they gate mains. ----
            audioT_sb = const.tile([128, PER], f16)
            waudioT_sb = const.tile([128, HSIZE], f16)
            baudio_sb = const.tile([128, HC], f32)
            wgT_sb = const.tile([128, HC, MSIZE], f16)
            wvideoT8_sb = const.tile([128, K8 // 128, HSIZE], f8)
            nc.gpsimd.dma_start(out=wvideoT8_sb, in_=wvideoT8_d)
            w16h_sb = const.tile([128, K16C, HSIZE], f16)
            nc.gpsimd.dma_start(out=w16h_sb, in_=w16h_d)
            bvideo_sb = const.tile([128, HC], f32)
            nc.gpsimd.dma_start(out=bvideo_sb, in_=bvideo_d)
            wvT_sb = const.tile([128, HC, MSIZE], f16)
            nc.gpsimd.dma_start(out=wvT_sb, in_=wvT_d)
            wcomb_sb = const.tile([128, KC, MSIZE], f16)
            nc.gpsimd.dma_start(out=wcomb_sb, in_=wcomb_d)
            whT_sb = const.tile([112, 1], f32)
            nc.gpsimd.dma_start(out=whT_sb, in_=whT_d)
            cbias_sb = const.tile([112, 1], f32)
            nc.gpsimd.dma_start(out=cbias_sb, in_=cbias_d)
            ones_m = const.tile([112, 128], f32)
            nc.vector.memset(ones_m, 1.0)
            # W_h replicated across 128 free cols, on partitions 0-47 AND
            # 64-111 (rows 48-63 zero) for the two row-tiled z matmuls
            whB_sb = const.tile([112, 128], f16)
            nc.scalar.mul(out=whB_sb, in_=ones_m, mul=whT_sb)
            # HAM warm-up: keep the PE busy during the initial DMA fill so the
            # clock gate is at 8/8 (2.4 GHz) before the real matmuls arrive
            warm_sb = const.tile([128, 64], f16)
            nc.vector.memset(warm_sb.bitcast(f32), 0.0)
            ones_f32 = const.tile([1, 128], f32)
            nc.vector.memset(ones_f32, 1.0)
            ones48 = const.tile([1, MSIZE], f16)
            nc.vector.tensor_copy(out=ones48, in_=ones_f32[:, :MSIZE])

            # persistent accumulators
            cT_acc = const.tile([128, KC, PER], f16)
            interflat_all = const.tile([1, R], f16)

            with (
                tc.tile_pool(name="vt", bufs=3) as vtp,
                tc.tile_pool(name="vt8", bufs=3) as vt8p,
                tc.tile_pool(name="vrelu", bufs=2) as vrp,
                tc.tile_pool(name="tanhp", bufs=2) as thp,
                tc.tile_pool(name="ezb", bufs=2) as ezp,
                tc.tile_pool(name="tree", bufs=2) as trp,
                tc.tile_pool(name="mm_ps", bufs=2, space="PSUM") as mm_ps,
                tc.tile_pool(name="ct_ps", bufs=1, space="PSUM") as ct_ps,
                tc.tile_pool(name="z_ps", bufs=1, space="PSUM") as z_ps,
            ):
                vt_t, vt8_t, vr_t, th_t, ez_t = {}, {}, {}, {}, {}

                def alloc_sb(sb):
                    # allocate the superblock's tiles and issue their DMAs.
                    # sb 0 is chunked (small first transfers so the first
                    # matmul starts early, fp8 first since it heads the PSUM
                    # chain); later sbs stream whole on the sync ring.
                    vt_t[sb] = vtp.tile([128, KC, SUPER], f16, tag="vt",
                                        name=f"vt_{sb}")
                    vt8_t[sb] = vt8p.tile([128, K8 // 128, SUPER], f8,
                                          tag="vt8", name=f"vt8_{sb}")
                    if sb == 0:
                        for i in range(K8 // 128):
                            nc.sync.dma_start(
                                out=vt8_t[sb][:, i, 0:DSUB],
                                in_=vT8_d[i * 128 : (i + 1) * 128, 0:DSUB],
                            )
                        # the two fp16 chunks the first mains chain needs go
                        # on the scalar ring (ahead of the audio weights) so
                        # they issue in parallel with the sync-ring fp8 chunks
                        for k in range(K8 // 128, KC):
                            nc.scalar.dma_start(
                                out=vt_t[sb][:, k, 0:DSUB],
                                in_=vT_d[k * 128 : (k + 1) * 128, 0:DSUB],
                            )
                        # strict need-order on the sync ring: pair-1 data
                        # (cc=1) before the k0/k1 first chunks (only needed
                        # by content(0), two pairs later), then cc=2.  This
                        # keeps the first mains' critical transfers (vt8 +
                        # scalar-ring k2/k3 + gpsimd-ring weights) from
                        # queuing behind ~2MB of bulk.
                        nc.sync.dma_start(
                            out=vt8_t[sb][:, :, DSUB : 2 * DSUB],
                            in_=vT8_d[:, DSUB : 2 * DSUB].rearrange(
                                "(c p) n -> p c n", p=128
                            ),
                        )
                        nc.sync.dma_start(
                            out=vt_t[sb][:, :, DSUB : 2 * DSUB],
                            in_=vT_d[:, DSUB : 2 * DSUB].rearrange(
                                "(c p) n -> p c n", p=128
                            ),
                        )
                        for k in range(K8 // 128):
                            nc.sync.dma_start(
                                out=vt_t[sb][:, k, 0:DSUB],
                                in_=vT_d[k * 128 : (k + 1) * 128, 0:DSUB],
                            )
                        nc.sync.dma_start(
                            out=vt8_t[sb][:, :, 2 * DSUB : PPS * DSUB],
                            in_=vT8_d[:, 2 * DSUB : PPS * DSUB].rearrange(
                                "(c p) n -> p c n", p=128
                            ),
                        )
                        nc.sync.dma_start(
                            out=vt_t[sb][:, :, 2 * DSUB : PPS * DSUB],
                            in_=vT_d[:, 2 * DSUB : PPS * DSUB].rearrange(
                                "(c p) n -> p c n", p=128
                            ),
                        )
                    else:
                        nc.sync.dma_start(
                            out=vt8_t[sb],
                            in_=vT8_d[:, sb * SUPER : (sb + 1) * SUPER].rearrange(
                                "(c p) n -> p c n", p=128
                            ),
                        )
                        nc.sync.dma_start(
                            out=vt_t[sb],
                            in_=vT_d[:, sb * SUPER : (sb + 1) * SUPER].rearrange(
                                "(c p) n -> p c n", p=128
                            ),
                        )
                    vr_t[sb] = vrp.tile([128, HC, SUPER], f16, tag="vrelu",
                                        name=f"vrelu_{sb}")
                    # tanh halves: col half A on partitions 0-47, half B on
                    # 64-111, both at free offset 512p (same ACT op)
                    th_t[sb] = thp.tile([112, SUPER // 2], f16, tag="tanhc",
                                        name=f"tanhc_{sb}")
                    ez_t[sb] = ezp.tile([128, SUPER], f16, tag="ezb",
                                        name=f"ezb_{sb}")

                # issue the first superblock's DMAs BEFORE the warm burst so
                # the sync/scalar sequencers start moving data immediately;
                # the warm matmuls then cover exactly the remaining fill time.
                # The audio-path loads follow on the scalar ring (needed a
                # few us later than the first mains chunks).
                alloc_sb(0)
                nc.scalar.dma_start(out=audioT_sb, in_=audioT_d)
                nc.scalar.dma_start(out=waudioT_sb, in_=waudioT_d)
                nc.scalar.dma_start(out=baudio_sb, in_=baudio_d)
                nc.scalar.dma_start(out=wgT_sb, in_=wgT_d)

                warm_ps = mm_ps.tile([64, 64], f32, tag="v_ps", name="warm_ps")

                def warm_burst(n):
                    for _ in range(n):
                        nc.tensor.matmul(
                            warm_ps, warm_sb[:, :64], warm_sb, start=True, stop=True
                        )

                warm_burst(58)

                def emit_audio():
                    # a.T = relu(W_audio.T^T @ audio.T + b_audio); runs on the
                    # PE right after the first mains pair (its DMAs land much
                    # earlier than the video stream)
                    aT_sb = const.tile([128, HC, PER], f16)
                    for m in range(HC):
                        a_ps = mm_ps.tile([128, PER], f32, tag="v_ps",
                                          name=f"a_ps_{m}")
                        nc.tensor.matmul(
                            a_ps,
                            waudioT_sb[:, m * 128 : (m + 1) * 128],
                            audioT_sb,
                            start=True,
                            stop=True,
                        )
                        nc.scalar.activation(
                            out=aT_sb[:, m, :], in_=a_ps, func=AF.Relu,
                            bias=baudio_sb[:, m : m + 1],
                        )
                    # inter[bt, m] = a @ W_g.T, natural layout for a flat write
                    inter_sb = const.tile([128, PER // 128, MSIZE], f16)
                    for t in range(PER // 128):
                        i_ps = mm_ps.tile([128, MSIZE], f32, tag="v_ps",
                                          name=f"i_ps_{t}")
                        for k in range(HC):
                            nc.tensor.matmul(
                                i_ps,
                                aT_sb[:, k, t * 128 : (t + 1) * 128],
                                wgT_sb[:, k, :],
                                start=(k == 0),
                                stop=(k == HC - 1),
                            )
                        nc.scalar.copy(out=inter_sb[:, t, :], in_=i_ps)
                    # flatten inter [bt, m] row-major into a single-partition
                    # row via SBUF->SBUF DMA (no HBM roundtrip)
                    for t in range(PER // 128):
                        nc.gpsimd.dma_start(
                            out=interflat_all[
                                :, t * 128 * MSIZE : (t + 1) * 128 * MSIZE
                            ],
                            in_=inter_sb[:, t, :],
                        )

                def emit_mains(q, ms):
                    sb, p = divmod(q, PPS)
                    # prefetch the next superblock's tiles one pair earlier
                    # than first use (bufs=3 pools absorb the extra lifetime)
                    if p == 1 and ms[0] == 0 and sb + 1 < NSB:
                        alloc_sb(sb + 1)
                    c0 = p * DSUB
                    for m in ms:
                        # both 512-col halves of this m-chunk accumulate into
                        # one 2-bank PSUM tile -> ONE fused |pre+b| of 1024
                        # cols.  Chain per half: 1 fp8 DoubleRow MM (rows
                        # 0:256, 2 rows/cycle) + 2 fp16 MMs (rows 256:512).
                        # weight-major order: each of the 3 stationary weights
                        # (fp16 a, fp16 b, fp8 pair) is loaded ONCE and runs
                        # both 512-col halves back-to-back.  The fp16 chunks
                        # go FIRST and the fp8 DoubleRow chunk LAST, so the
                        # 256-col DR weight load (213ns, no pull-ahead) hides
                        # under the two preceding 213ns fp16 matmuls, and the
                        # next m's 128-col fp16 load hides under the DR pair.
                        v_ps = mm_ps.tile([128, 2, SUB], f32, tag="v_ps",
                                          name=f"v_ps_{q}_{m}")
                        dr_first = q == 0
                        if dr_first:
                            for j in range(2):
                                nc.tensor.matmul(
                                    v_ps[:, j, :],
                                    wvideoT8_sb[:, :, m * 128 : (m + 1) * 128],
                                    vt8_t[sb][:, :, c0 + j * SUB : c0 + (j + 1) * SUB],
                                    start=True,
                                    stop=False,
                                    perf_mode=DR,
                                )
                        for c in range(K16C):
                            for j in range(2):
                                nc.tensor.matmul(
                                    v_ps[:, j, :],
                                    w16h_sb[:, c, m * 128 : (m + 1) * 128],
                                    vt_t[sb][:, K8 // 128 + c,
                                             c0 + j * SUB : c0 + (j + 1) * SUB],
                                    start=(c == 0 and not dr_first),
                                    stop=(dr_first and c == K16C - 1),
                                )
                        if not dr_first:
                            for j in range(2):
                                nc.tensor.matmul(
                                    v_ps[:, j, :],
                                    wvideoT8_sb[:, :, m * 128 : (m + 1) * 128],
                                    vt8_t[sb][:, :, c0 + j * SUB : c0 + (j + 1) * SUB],
                                    start=False,
                                    stop=True,
                                    perf_mode=DR,
                                )
                        nc.scalar.activation(
                            out=vr_t[sb][:, m, c0 : c0 + DSUB].rearrange(
                                "p (a b) -> p a b", a=2
                            ),
                            in_=v_ps, func=AF.Abs,
                            bias=bvideo_sb[:, m : m + 1],
                            scale=1.0 / PRE_SCALE,
                        )

                def emit_content(q):
                    # two 512-col half chains packed onto array col groups
                    # 0-47 / 64-111; the chains run concurrently on the PE.
                    # chain: inter broadcast + 4x (0.5*W_v.T : |pre+b|)
                    #        + 4x (Wcomb : vT)   [Wcomb = 0.5*W_video.T@W_v.T]
                    sb, p = divmod(q, PPS)
                    c0 = p * DSUB
                    r0 = sb * SUPER
                    ct = ct_ps.tile([128, SUB], f32, tag="ct", name=f"ct_{q}")
                    ctA = ct[0:MSIZE, :]
                    ctB = ct[64 : 64 + MSIZE, :]
                    nc.tensor.matmul(
                        ctA, ones48,
                        interflat_all[:, r0 + c0 : r0 + c0 + SUB],
                        start=True, stop=False,
                    )
                    nc.tensor.matmul(
                        ctB, ones48,
                        interflat_all[:, r0 + c0 + SUB : r0 + c0 + DSUB],
                        start=True, stop=False,
                    )
                    for k in range(HC):
                        nc.tensor.matmul(
                            ctA, wvT_sb[:, k, :], vr_t[sb][:, k, c0 : c0 + SUB],
                            start=False, stop=False,
                        )
                        nc.tensor.matmul(
                            ctB, wvT_sb[:, k, :],
                            vr_t[sb][:, k, c0 + SUB : c0 + DSUB],
                            start=False, stop=False,
                        )
                    for k in range(KC):
                        nc.tensor.matmul(
                            ctA, wcomb_sb[:, k, :], vt_t[sb][:, k, c0 : c0 + SUB],
                            start=False, stop=(k == KC - 1),
                        )
                        nc.tensor.matmul(
                            ctB, wcomb_sb[:, k, :],
                            vt_t[sb][:, k, c0 + SUB : c0 + DSUB],
                            start=False, stop=(k == KC - 1),
                        )
                    # one fused tanh over partitions 0-111 (rows 48-63 junk);
                    # cbias = 0.5*(b_video @ W_v.T) rides the ACT bias
                    nc.scalar.activation(
                        out=th_t[sb][0:112, p * SUB : (p + 1) * SUB],
                        in_=ct[0:112, :], func=AF.Tanh,
                        bias=cbias_sb,
                    )

                def emit_zmm(q):
                    sb, p = divmod(q, PPS)
                    # two row-tiled z matmuls (array rows 0-47 / 64-111) into
                    # the two banks of one PSUM tile, concurrent on the PE
                    zt = z_ps.tile([128, 2, SUB], f32, tag="zt", name=f"z_{q}")
                    nc.tensor.matmul(
                        zt[:, 0, :], whB_sb[0:MSIZE, :],
                        th_t[sb][0:MSIZE, p * SUB : (p + 1) * SUB],
                        start=True, stop=True,
                    )
                    nc.tensor.matmul(
                        zt[:, 1, :], whB_sb[64 : 64 + MSIZE, :],
                        th_t[sb][64 : 64 + MSIZE, p * SUB : (p + 1) * SUB],
                        start=True, stop=True,
                    )
                    return zt

                def emit_expmul(q, zt, j=None):
                    # exp + in-place weighted multiply; j=None does the full
                    # DSUB in one fused exp + one mul, j=0/1 does one half
                    sb, p = divmod(q, PPS)
                    c0 = p * DSUB
                    if j is None:
                        nc.scalar.activation(
                            out=ez_t[sb][:, c0 : c0 + DSUB].rearrange(
                                "p (a b) -> p a b", a=2
                            ),
                            in_=zt, func=AF.Exp,
                        )
                        nc.vector.tensor_mul(
                            vt_t[sb][:, :, c0 : c0 + DSUB],
                            vt_t[sb][:, :, c0 : c0 + DSUB],
                            ez_t[sb][:, c0 : c0 + DSUB]
                            .unsqueeze(1)
                            .broadcast_to([128, KC, DSUB]),
                        )
                    else:
                        cj = c0 + j * SUB
                        nc.scalar.activation(
                            out=ez_t[sb][:, cj : cj + SUB], in_=zt[:, j, :],
                            func=AF.Exp,
                        )
                        nc.vector.tensor_mul(
                            vt_t[sb][:, :, cj : cj + SUB],
                            vt_t[sb][:, :, cj : cj + SUB],
                            ez_t[sb][:, cj : cj + SUB]
                            .unsqueeze(1)
                            .broadcast_to([128, KC, SUB]),
                        )

                def emit_score(q):
                    emit_expmul(q, emit_zmm(q))

                def emit_finalize(sb, g0f, ng, dma=None, ring=None):
                    # reduce cols [g0f*48, (g0f+ng)*48) of superblock sb
                    # (units of 48-col sample groups, local to sb).
                    # dma=(gd0, ngd) flushes that global group range of
                    # cT_acc to DRAM.
                    ch = g0f * MSIZE
                    g0 = sb * GPS + g0f
                    lp = nc.allow_low_precision(
                        reason="fp16 group sums; fp32 internal accum"
                    )
                    lp.__enter__()
                    tree = trp.tile([128, KC, FGRP, MSIZE // 2], f16, tag="tree",
                                    name=f"tree_{sb}_{g0f}")
                    wv = vt_t[sb][:, :, ch : ch + ng * MSIZE].rearrange(
                        "p c (g n) -> p c g n", n=MSIZE
                    )
                    tr = tree[:, :, :ng, :]
                    nc.vector.tensor_add(
                        tr, wv[:, :, :, : MSIZE // 2], wv[:, :, :, MSIZE // 2 :]
                    )
                    nc.vector.tensor_add(
                        tr[:, :, :, : MSIZE // 4],
                        tr[:, :, :, : MSIZE // 4],
                        tr[:, :, :, MSIZE // 4 :],
                    )
                    nc.vector.tensor_add(
                        tr[:, :, :, : MSIZE // 8],
                        tr[:, :, :, : MSIZE // 8],
                        tr[:, :, :, MSIZE // 8 : MSIZE // 4],
                    )
                    nc.vector.reduce_sum(
                        out=cT_acc[:, :, g0 : g0 + ng],
                        in_=tr[:, :, :, : MSIZE // 8],
                        axis=AX.X,
                    )
                    lp.__exit__(None, None, None)
                    if dma is not None:
                        gd0, ngd = dma
                        (ring or nc.gpsimd).dma_start(
                            out=cT_d[:, gd0 : gd0 + ngd].rearrange(
                                "(c p) n -> p c n", p=128
                            ),
                            in_=cT_acc[:, :, gd0 : gd0 + ngd],
                        )

                def emit_ezrow(sb, ring=None):
                    (ring or nc.gpsimd).dma_start(
                        out=ezrow_d[:, sb * SUPER : (sb + 1) * SUPER],
                        in_=ez_t[sb][0:1, :],
                    )

                # software-pipelined emission with a ONE-pair lag, interleaved
                # at half-pair granularity so no consumer head-of-line-blocks
                # the PE queue (see v1 docstring for the full rationale).
                lsb = NSB - 1

                def emit_fins(qq):
                    sb2, p2 = divmod(qq, PPS)
                    if p2 == 0:
                        emit_finalize(sb2, 0, FGRP)
                    elif p2 == 1:
                        emit_finalize(sb2, FGRP, FGRP)
                        emit_finalize(sb2, 2 * FGRP, 10)
                    else:
                        emit_finalize(sb2, 42, 11)
                        emit_finalize(sb2, 53, 11, dma=(sb2 * GPS, GPS))
                        emit_ezrow(sb2)

                def emit_consume(qq):
                    sb2, p2 = divmod(qq, PPS)
                    if sb2 < lsb:
                        emit_score(qq)
                        emit_fins(qq)
                        return
                    # last superblock: exp/mul per 512-col half, with the
                    # largest group-aligned chunk each half unlocks
                    zt = emit_zmm(qq)
                    emit_expmul(qq, zt, j=0)
                    if p2 == 0:
                        emit_finalize(lsb, 0, 10)
                    elif p2 == 1:
                        emit_finalize(lsb, 21, 11)
                    else:
                        emit_finalize(lsb, 42, 11, dma=(lsb * GPS + 42, 11))
                    emit_expmul(qq, zt, j=1)
                    if p2 == 0:
                        emit_finalize(lsb, 10, 11)
                    elif p2 == 1:
                        emit_finalize(lsb, 32, 10, dma=(lsb * GPS, 42))
                    else:
                        # ezrow first (only needs the exps, long done); the
                        # final finalize is split so the very last cT DMA is
                        # small and issues ~1us earlier.  The three tail DMAs
                        # issue on three different rings (the ~0.9us hwdge
                        # descriptor-gen cost overlaps instead of serializing)
                        emit_ezrow(lsb, ring=nc.scalar)
                        emit_finalize(lsb, 53, 6, dma=(lsb * GPS + 53, 6),
                                      ring=nc.sync)
                        emit_finalize(lsb, 59, 5, dma=(lsb * GPS + 59, 5),
                                      ring=nc.scalar)

                emit_mains(0, (0, 1))
                emit_mains(0, (2, 3))
                emit_audio()
                for q in range(1, NPAIR):
                    emit_mains(q, (0, 1))
                    emit_content(q - 1)
                    # consume between m2 and m3: content(q-1)'s tanh drains
                    # during m2, so the z matmuls slot in without stalling
                    # and the DVE work starts ~1.7us earlier
                    emit_mains(q, (2,))
                    emit_consume(q - 1)
                    emit_mains(q, (3,))
                emit_content(NPAIR - 1)
                emit_consume(NPAIR - 1)

    nc.compile()
    return nc


def _prep_in_maps(inputs):
    import ml_dtypes

    E4M3 = ml_dtypes.float8_e4m3

    audio = np.ascontiguousarray(np.asarray(inputs["audio"], np.float32))
    video = np.ascontiguousarray(np.asarray(inputs["video"], np.float32))

    def dev_chunks(w):  # [C*128, X] -> [128, C, X] (partition-major chunks)
        a = np.asarray(w)
        return np.ascontiguousarray(a.reshape(-1, 128, a.shape[-1]).transpose(1, 0, 2))

    W_video = np.asarray(inputs["W_video"], np.float32)   # [h, v]
    W_videoT = W_video.T                                  # [v, h]
    # fp8 DoubleRow half: rows 0:K8, pre-scaled by W8_SCALE
    WvideoT8 = np.ascontiguousarray(
        (W_videoT[:K8] * W8_SCALE).astype(E4M3)
        .reshape(K8 // 128, 128, HSIZE).transpose(1, 0, 2)
    )
    # fp16 half: rows K8:, pre-scaled by PRE_SCALE (power of 2, exact)
    W16h = np.ascontiguousarray(
        (W_videoT[K8:] * PRE_SCALE).astype(np.float16)
        .reshape(-1, 128, HSIZE).transpose(1, 0, 2)
    )
    WaudioT = np.ascontiguousarray(np.asarray(inputs["W_audio"], np.float32).T.astype(np.float16))
    WgT = dev_chunks(np.asarray(inputs["W_g"], np.float32).T.astype(np.float16))
    W_v = np.asarray(inputs["W_v"], np.float32)
    WvT = dev_chunks((0.5 * W_v.T).astype(np.float16))
    # Wcomb = 0.5 * W_video.T @ W_v.T, computed exactly in float64 on host
    Wcomb = dev_chunks(
        (0.5 * (W_videoT.astype(np.float64) @ W_v.T.astype(np.float64)))
        .astype(np.float16)
    )
    wh = np.asarray(inputs["W_h"], np.float32).T  # [48, 1]
    WhT = np.zeros((112, 1), np.float32)
    WhT[0:MSIZE] = wh
    WhT[64 : 64 + MSIZE] = wh
    WhT = np.ascontiguousarray(WhT)
    b_video = np.asarray(inputs["b_video"], np.float32)
    b_videoT = np.ascontiguousarray(b_video.reshape(-1, 128).T)
    b_audio = np.ascontiguousarray(
        np.asarray(inputs["b_audio"], np.float32).reshape(-1, 128).T
    )
    # cbias = 0.5 * (b_video @ W_v.T), on partitions 0-47 and 64-111
    cb = 0.5 * (b_video @ W_v.T)
    cbias = np.zeros((112, 1), np.float32)
    cbias[0:MSIZE, 0] = cb
    cbias[64 : 64 + MSIZE, 0] = cb
    cbias = np.ascontiguousarray(cbias)

    a2 = audio.reshape(BT, ASIZE).astype(np.float16)
    v2f = video.reshape(BT, MSIZE, VSIZE)
    v2h = v2f.astype(np.float16)
    in_maps = []
    for c in range(NCORES):
        sl = slice(c * PER, (c + 1) * PER)
        vT = np.ascontiguousarray(v2h[sl].reshape(R, VSIZE).T)
        # fp8 copy of contraction rows 0:K8, scaled by V8_SCALE, quantized
        # from the fp32 source (matches the accuracy sim)
        vT8 = np.ascontiguousarray(
            (v2f[sl].reshape(R, VSIZE).T[:K8] * V8_SCALE).astype(E4M3)
        )
        audioT = np.ascontiguousarray(a2[sl].T)
        in_maps.append(
            {
                "vT": vT,
                "vT8": vT8,
                "audioT": audioT,
                "WvideoT8": WvideoT8,
                "W16h": W16h,
                "WaudioT": WaudioT,
                "WgT": WgT,
                "WvT": WvT,
                "Wcomb": Wcomb,
                "WhT": WhT,
                "b_video": b_videoT,
                "b_audio": b_audio,
                "cbias": cbias,
            }
        )
    return in_maps


def _run(inputs, trace=False, **spmd_kwargs):
    from concourse.bass_utils import run_bass_kernel_spmd

    if "nc" not in _cached:
        _cached["nc"] = _build_nc()
    nc = _cached["nc"]
    in_maps = _prep_in_maps(inputs)
    res = run_bass_kernel_spmd(
        nc, in_maps, core_ids=list(range(NCORES)), trace=trace, **spmd_kwargs
    )
    def _part(r):
        denom = r["ezrow"].astype(np.float32).reshape(PER, MSIZE).sum(axis=1)
        return (r["cT"].astype(np.float32) / denom[None, :]).T

    parts = [_part(r) for r in res.results]
    out = np.concatenate(parts, axis=0).reshape(B, T, VSIZE)
    return np.ascontiguousarray(out.astype(np.float32)), res


def kernel(**inputs):
    out, _ = _run(inputs, trace=False)
    return out


# revision 23
# speedup vs baseline: 1.1942x; 1.0025x over previous
"""Trainium2 Bass kernel for the audio-visual attention model (v2: fp8 hybrid).

Math (per (b,t) sample, BT = 32*64 = 2048 of them):
    V   = video[b,t]                              # [48, 512]
    v   = relu(V @ W_video.T + b_video)           # [48, 512]
    a   = relu(audio[b,t] @ W_audio.T + b_audio)  # [512]
    inter   = a @ W_g.T                           # [48]
    content = v @ W_v.T + inter[:, None]          # [48, 48]
    z   = tanh(content) @ W_h.T                   # [48]
    alpha = softmax(z)
    out = alpha @ V                               # [512]

v2 changes vs the fp16 baseline (139us):
  * relu split:  relu(x) = (x + |x|)/2, so
        content = 0.5*|pre+b| @ W_v.T + 0.5*V @ (W_video.T @ W_v.T)
                  + 0.5*(b @ W_v.T) + inter
    The linear half rides an exact host-precomputed [512,48] Wcomb in fp16;
    only the |pre| half carries main-matmul error.
  * mains k-hybrid: contraction rows 0:256 run as ONE fp8e4 DoubleRow matmul
    (2 rows/cycle), rows 256:512 as two fp16 matmuls.  Host pre-scales
    V*16 / W*32 (fp8) and W*512 (fp16) so the PSUM holds 512*pre; the Abs
    activation rescales by 1/512.  Predicted rel err 1.47e-2 (sim), vs the
    2e-2 gate; pure-fp8 mains measure 2.9e-2 and are not usable.

Strategy: data-parallel over BT across 8 cores (256 samples each, R = 256*48
= 12288 video rows per core).  The host pre-transposes the video shard to
V.T [512, 12288] fp16 plus an fp8 copy of rows 0:256, pre-arranges weights
into device layouts, and runs the matmul chain per superblock as in v1:
vT->|pre| (PE+ACT), content.T via col-tiled concurrent chains (PE), tanh
(ACT), z row-tiled (PE), exp (ACT), weighted mul + halving-tree reduce
(DVE), outputs streamed per chunk on the gpsimd DMA ring.
"""

import numpy as np

# Problem constants (hardcoded per harness contract).
B, T = 32, 64
ASIZE, VSIZE, HSIZE, MSIZE = 128, 512, 512, 48
NCORES = 8
BT = B * T                     # 2048
PER = BT // NCORES             # 256 samples per core
R = PER * MSIZE                # 12288 video rows per core
SUPER = 3072                   # rows per superblock (64 groups of 48)
NSB = R // SUPER               # 4 superblocks
SUB = 512                      # matmul moving-dim block (PSUM bank limit)
DSUB = 2 * SUB                 # 1024-col double block for content/score
NPAIR = R // DSUB              # 12 double blocks
PPS = SUPER // DSUB            # 3 double blocks per superblock
GPS = SUPER // MSIZE           # 64 sample groups per superblock
FGRP = 16                      # groups per finalize chunk

K8 = 256                       # contraction rows 0:K8 in fp8 DoubleRow
V8_SCALE = 16.0
W8_SCALE = 32.0
PRE_SCALE = V8_SCALE * W8_SCALE  # PSUM holds PRE_SCALE * pre

_cached = {}


def _build_nc():
    import concourse.bacc as bacc
    import concourse.mybir as mybir
    import concourse.tile as tile

    f32 = mybir.dt.float32
    f16 = mybir.dt.float16
    f8 = mybir.dt.float8e4
    AF = mybir.ActivationFunctionType
    AX = mybir.AxisListType
    DR = mybir.MatmulPerfMode.DoubleRow

    nc = bacc.Bacc(
        "TRN2",
        target_bir_lowering=False,
        debug=False,
        enable_asserts=False,
        num_devices=NCORES,
    )

    # ---- DRAM I/O ----
    vT_d = nc.dram_tensor("vT", [VSIZE, R], f16, kind="ExternalInput").ap()
    vT8_d = nc.dram_tensor("vT8", [K8, R], f8, kind="ExternalInput").ap()
    audioT_d = nc.dram_tensor("audioT", [ASIZE, PER], f16, kind="ExternalInput").ap()
    wvideoT8_d = nc.dram_tensor("WvideoT8", [128, K8 // 128, HSIZE], f8, kind="ExternalInput").ap()
    w16h_d = nc.dram_tensor("W16h", [128, (VSIZE - K8) // 128, HSIZE], f16, kind="ExternalInput").ap()
    waudioT_d = nc.dram_tensor("WaudioT", [ASIZE, HSIZE], f16, kind="ExternalInput").ap()
    wgT_d = nc.dram_tensor("WgT", [128, HSIZE // 128, MSIZE], f16, kind="ExternalInput").ap()
    wvT_d = nc.dram_tensor("WvT", [128, HSIZE // 128, MSIZE], f16, kind="ExternalInput").ap()
    wcomb_d = nc.dram_tensor("Wcomb", [128, VSIZE // 128, MSIZE], f16, kind="ExternalInput").ap()
    whT_d = nc.dram_tensor("WhT", [112, 1], f32, kind="ExternalInput").ap()
    bvideo_d = nc.dram_tensor("b_video", [128, HSIZE // 128], f32, kind="ExternalInput").ap()
    baudio_d = nc.dram_tensor("b_audio", [128, HSIZE // 128], f32, kind="ExternalInput").ap()
    cbias_d = nc.dram_tensor("cbias", [112, 1], f32, kind="ExternalInput").ap()
    cT_d = nc.dram_tensor("cT", [VSIZE, PER], f16, kind="ExternalOutput").ap()
    # ez row 0 per sample-column; the host computes denom = group-sums of 48
    # in fp32 (cheaper and more accurate than on-device fp16 reduces)
    ezrow_d = nc.dram_tensor("ezrow", [1, R], f16, kind="ExternalOutput").ap()

    KC = VSIZE // 128          # 4 v chunks (weighted mul / Wcomb contraction)
    K16C = (VSIZE - K8) // 128  # 2 fp16 mains chunks
    HC = HSIZE // 128          # 4 h chunks

    with tile.TileContext(nc) as tc:
        with (
            tc.tile_pool(name="const", bufs=1) as const,
        ):
            # ---- constants / weights.  Audio-path tensors go on the scalar
            # ring (they gate the first PE work); the big main-loop weights go
            # on the gpsimd ring, wvideoT8/w16h first -- they gate mains. ----
            audioT_sb = const.tile([128, PER], f16)
            waudioT_sb = const.tile([128, HSIZE], f16)
            baudio_sb = const.tile([128, HC], f32)
            wgT_sb = const.tile([128, HC, MSIZE], f16)
            wvideoT8_sb = const.tile([128, K8 // 128, HSIZE], f8)
            nc.gpsimd.dma_start(out=wvideoT8_sb, in_=wvideoT8_d)
            w16h_sb = const.tile([128, K16C, HSIZE], f16)
            nc.gpsimd.dma_start(out=w16h_sb, in_=w16h_d)
            bvideo_sb = const.tile([128, HC], f32)
            nc.gpsimd.dma_start(out=bvideo_sb, in_=bvideo_d)
            wvT_sb = const.tile([128, HC, MSIZE], f16)
            nc.gpsimd.dma_start(out=wvT_sb, in_=wvT_d)
            wcomb_sb = const.tile([128, KC, MSIZE], f16)
            nc.gpsimd.dma_start(out=wcomb_sb, in_=wcomb_d)
            whT_sb = const.tile([112, 1], f32)
            nc.gpsimd.dma_start(out=whT_sb, in_=whT_d)
            cbias_sb = const.tile([112, 1], f32)
            nc.gpsimd.dma_start(out=cbias_sb, in_=cbias_d)
            ones_m = const.tile([112, 128], f32)
            nc.vector.memset(ones_m, 1.0)
            # W_h replicated across 128 free cols, on partitions 0-47 AND
            # 64-111 (rows 48-63 zero) for the two row-tiled z matmuls
            whB_sb = const.tile([112, 128], f16)
            nc.scalar.mul(out=whB_sb, in_=ones_m, mul=whT_sb)
            # HAM warm-up: keep the PE busy during the initial DMA fill so the
            # clock gate is at 8/8 (2.4 GHz) before the real matmuls arrive
            warm_sb = const.tile([128, 64], f16)
            nc.vector.memset(warm_sb.bitcast(f32), 0.0)
            ones_f32 = const.tile([1, 128], f32)
            nc.vector.memset(ones_f32, 1.0)
            ones48 = const.tile([1, MSIZE], f16)
            nc.vector.tensor_copy(out=ones48, in_=ones_f32[:, :MSIZE])

            # persistent accumulators
            cT_acc = const.tile([128, KC, PER], f16)
            interflat_all = const.tile([1, R], f16)

            with (
                tc.tile_pool(name="vt", bufs=3) as vtp,
                tc.tile_pool(name="vt8", bufs=3) as vt8p,
                tc.tile_pool(name="vrelu", bufs=2) as vrp,
                tc.tile_pool(name="tanhp", bufs=2) as thp,
                tc.tile_pool(name="ezb", bufs=2) as ezp,
                tc.tile_pool(name="tree", bufs=2) as trp,
                tc.tile_pool(name="mm_ps", bufs=2, space="PSUM") as mm_ps,
                tc.tile_pool(name="ct_ps", bufs=1, space="PSUM") as ct_ps,
                tc.tile_pool(name="z_ps", bufs=1, space="PSUM") as z_ps,
            ):
                vt_t, vt8_t, vr_t, th_t, ez_t = {}, {}, {}, {}, {}

                def alloc_sb(sb):
                    # allocate the superblock's tiles and issue their DMAs.
                    # sb 0 is chunked (small first transfers so the first
                    # matmul starts early, fp8 first since it heads the PSUM
                    # chain); later sbs stream whole on the sync ring.
                    vt_t[sb] = vtp.tile([128, KC, SUPER], f16, tag="vt",
                                        name=f"vt_{sb}")
                    vt8_t[sb] = vt8p.tile([128, K8 // 128, SUPER], f8,
                                          tag="vt8", name=f"vt8_{sb}")
                    if sb == 0:
                        for i in range(K8 // 128):
                            nc.sync.dma_start(
                                out=vt8_t[sb][:, i, 0:DSUB],
                                in_=vT8_d[i * 128 : (i + 1) * 128, 0:DSUB],
                            )
                        # the two fp16 chunks the first mains chain needs go
                        # on the scalar ring (ahead of the audio weights) so
                        # they issue in parallel with the sync-ring fp8 chunks
                        for k in range(K8 // 128, KC):
                            nc.scalar.dma_start(
                                out=vt_t[sb][:, k, 0:DSUB],
                                in_=vT_d[k * 128 : (k + 1) * 128, 0:DSUB],
                            )
                        # strict need-order on the sync ring: pair-1 data
                        # (cc=1) before the k0/k1 first chunks (only needed
                        # by content(0), two pairs later), then cc=2.  This
                        # keeps the first mains' critical transfers (vt8 +
                        # scalar-ring k2/k3 + gpsimd-ring weights) from
                        # queuing behind ~2MB of bulk.
                        nc.sync.dma_start(
                            out=vt8_t[sb][:, :, DSUB : 2 * DSUB],
                            in_=vT8_d[:, DSUB : 2 * DSUB].rearrange(
                                "(c p) n -> p c n", p=128
                            ),
                        )
                        nc.sync.dma_start(
                            out=vt_t[sb][:, :, DSUB : 2 * DSUB],
                            in_=vT_d[:, DSUB : 2 * DSUB].rearrange(
                                "(c p) n -> p c n", p=128
                            ),
                        )
                        for k in range(K8 // 128):
                            nc.sync.dma_start(
                                out=vt_t[sb][:, k, 0:DSUB],
                                in_=vT_d[k * 128 : (k + 1) * 128, 0:DSUB],
                            )
                        nc.sync.dma_start(
                            out=vt8_t[sb][:, :, 2 * DSUB : PPS * DSUB],
                            in_=vT8_d[:, 2 * DSUB : PPS * DSUB].rearrange(
                                "(c p) n -> p c n", p=128
                            ),
                        )
                        nc.sync.dma_start(
                            out=vt_t[sb][:, :, 2 * DSUB : PPS * DSUB],
                            in_=vT_d[:, 2 * DSUB : PPS * DSUB].rearrange(
                                "(c p) n -> p c n", p=128
                            ),
                        )
                    else:
                        nc.sync.dma_start(
                            out=vt8_t[sb],
                            in_=vT8_d[:, sb * SUPER : (sb + 1) * SUPER].rearrange(
                                "(c p) n -> p c n", p=128
                            ),
                        )
                        nc.sync.dma_start(
                            out=vt_t[sb],
                            in_=vT_d[:, sb * SUPER : (sb + 1) * SUPER].rearrange(
                                "(c p) n -> p c n", p=128
                            ),
                        )
                    vr_t[sb] = vrp.tile([128, HC, SUPER], f16, tag="vrelu",
                                        name=f"vrelu_{sb}")
                    # tanh halves: col half A on partitions 0-47, half B on
                    # 64-111, both at free offset 512p (same ACT op)
                    th_t[sb] = thp.tile([112, SUPER // 2], f16, tag="tanhc",
                                        name=f"tanhc_{sb}")
                    ez_t[sb] = ezp.tile([128, SUPER], f16, tag="ezb",
                                        name=f"ezb_{sb}")

                # issue the first superblock's DMAs BEFORE the warm burst so
                # the sync/scalar sequencers start moving data immediately;
                # the warm matmuls then cover exactly the remaining fill time.
                # The audio-path loads follow on the scalar ring (needed a
                # few us later than the first mains chunks).
                alloc_sb(0)
                nc.scalar.dma_start(out=audioT_sb, in_=audioT_d)
                nc.scalar.dma_start(out=waudioT_sb, in_=waudioT_d)
                nc.scalar.dma_start(out=baudio_sb, in_=baudio_d)
                nc.scalar.dma_start(out=wgT_sb, in_=wgT_d)

                warm_ps = mm_ps.tile([64, 64], f32, tag="v_ps", name="warm_ps")

                def warm_burst(n):
                    for _ in range(n):
                        nc.tensor.matmul(
                            warm_ps, warm_sb[:, :64], warm_sb, start=True, stop=True
                        )

                warm_burst(58)

                aT_sb = const.tile([128, HC, PER], f16)

                def emit_audio_a():
                    # a.T = relu(W_audio.T^T @ audio.T + b_audio); runs on the
                    # PE right after the first mains pair (its DMAs land much
                    # earlier than the video stream)
                    for m in range(HC):
                        a_ps = mm_ps.tile([128, PER], f32, tag="v_ps",
                                          name=f"a_ps_{m}")
                        nc.tensor.matmul(
                            a_ps,
                            waudioT_sb[:, m * 128 : (m + 1) * 128],
                            audioT_sb,
                            start=True,
                            stop=True,
                        )
                        nc.scalar.activation(
                            out=aT_sb[:, m, :], in_=a_ps, func=AF.Relu,
                            bias=baudio_sb[:, m : m + 1],
                        )

                def emit_audio_inter():
                    # inter[bt, m] = a @ W_g.T, natural layout for a flat
                    # write.  Emitted one half-pair later than the a-phase so
                    # mains(1,(0,1)) covers the ACT-queue wait for the relus.
                    # i_ps uses the ct PSUM pool: content(0) is its natural
                    # successor there, so no entanglement with the mains pool.
                    inter_sb = const.tile([128, PER // 128, MSIZE], f16)
                    for t in range(PER // 128):
                        i_ps = ct_ps.tile([128, MSIZE], f32, tag="ct",
                                          name=f"i_ps_{t}")
                        for k in range(HC):
                            nc.tensor.matmul(
                                i_ps,
                                aT_sb[:, k, t * 128 : (t + 1) * 128],
                                wgT_sb[:, k, :],
                                start=(k == 0),
                                stop=(k == HC - 1),
                            )
                        nc.scalar.copy(out=inter_sb[:, t, :], in_=i_ps)
                    # flatten inter [bt, m] row-major into a single-partition
                    # row via SBUF->SBUF DMA (no HBM roundtrip)
                    for t in range(PER // 128):
                        nc.gpsimd.dma_start(
                            out=interflat_all[
                                :, t * 128 * MSIZE : (t + 1) * 128 * MSIZE
                            ],
                            in_=inter_sb[:, t, :],
                        )

                def emit_mains(q, ms):
                    sb, p = divmod(q, PPS)
                    # prefetch the next superblock's tiles one pair earlier
                    # than first use (bufs=3 pools absorb the extra lifetime)
                    if p == 1 and ms[0] == 0 and sb + 1 < NSB:
                        alloc_sb(sb + 1)
                    c0 = p * DSUB
                    for m in ms:
                        # both 512-col halves of this m-chunk accumulate into
                        # one 2-bank PSUM tile -> ONE fused |pre+b| of 1024
                        # cols.  Chain per half: 1 fp8 DoubleRow MM (rows
                        # 0:256, 2 rows/cycle) + 2 fp16 MMs (rows 256:512).
                        # weight-major order: each of the 3 stationary weights
                        # (fp16 a, fp16 b, fp8 pair) is loaded ONCE and runs
                        # both 512-col halves back-to-back.  The fp16 chunks
                        # go FIRST and the fp8 DoubleRow chunk LAST, so the
                        # 256-col DR weight load (213ns, no pull-ahead) hides
                        # under the two preceding 213ns fp16 matmuls, and the
                        # next m's 128-col fp16 load hides under the DR pair.
                        v_ps = mm_ps.tile([128, 2, SUB], f32, tag="v_ps",
                                          name=f"v_ps_{q}_{m}")
                        dr_first = q == 0
                        if dr_first:
                            for j in range(2):
                                nc.tensor.matmul(
                                    v_ps[:, j, :],
                                    wvideoT8_sb[:, :, m * 128 : (m + 1) * 128],
                                    vt8_t[sb][:, :, c0 + j * SUB : c0 + (j + 1) * SUB],
                                    start=True,
                                    stop=False,
                                    perf_mode=DR,
                                )
                        for c in range(K16C):
                            for j in range(2):
                                nc.tensor.matmul(
                                    v_ps[:, j, :],
                                    w16h_sb[:, c, m * 128 : (m + 1) * 128],
                                    vt_t[sb][:, K8 // 128 + c,
                                             c0 + j * SUB : c0 + (j + 1) * SUB],
                                    start=(c == 0 and not dr_first),
                                    stop=(dr_first and c == K16C - 1),
                                )
                        if not dr_first:
                            for j in range(2):
                                nc.tensor.matmul(
                                    v_ps[:, j, :],
                                    wvideoT8_sb[:, :, m * 128 : (m + 1) * 128],
                                    vt8_t[sb][:, :, c0 + j * SUB : c0 + (j + 1) * SUB],
                                    start=False,
                                    stop=True,
                                    perf_mode=DR,
                                )
                        nc.scalar.activation(
                            out=vr_t[sb][:, m, c0 : c0 + DSUB].rearrange(
                                "p (a b) -> p a b", a=2
                            ),
                            in_=v_ps, func=AF.Abs,
                            bias=bvideo_sb[:, m : m + 1],
                            scale=1.0 / PRE_SCALE,
                        )

                def emit_content(q):
                    # two 512-col half chains packed onto array col groups
                    # 0-47 / 64-111; the chains run concurrently on the PE.
                    # chain: inter broadcast + 4x (0.5*W_v.T : |pre+b|)
                    #        + 4x (Wcomb : vT)   [Wcomb = 0.5*W_video.T@W_v.T]
                    sb, p = divmod(q, PPS)
                    c0 = p * DSUB
                    r0 = sb * SUPER
                    ct = ct_ps.tile([128, SUB], f32, tag="ct", name=f"ct_{q}")
                    ctA = ct[0:MSIZE, :]
                    ctB = ct[64 : 64 + MSIZE, :]
                    nc.tensor.matmul(
                        ctA, ones48,
                        interflat_all[:, r0 + c0 : r0 + c0 + SUB],
                        start=True, stop=False,
                    )
                    nc.tensor.matmul(
                        ctB, ones48,
                        interflat_all[:, r0 + c0 + SUB : r0 + c0 + DSUB],
                        start=True, stop=False,
                    )
                    for k in range(HC):
                        nc.tensor.matmul(
                            ctA, wvT_sb[:, k, :], vr_t[sb][:, k, c0 : c0 + SUB],
                            start=False, stop=False,
                        )
                        nc.tensor.matmul(
                            ctB, wvT_sb[:, k, :],
                            vr_t[sb][:, k, c0 + SUB : c0 + DSUB],
                            start=False, stop=False,
                        )
                    for k in range(KC):
                        nc.tensor.matmul(
                            ctA, wcomb_sb[:, k, :], vt_t[sb][:, k, c0 : c0 + SUB],
                            start=False, stop=(k == KC - 1),
                        )
                        nc.tensor.matmul(
                            ctB, wcomb_sb[:, k, :],
                            vt_t[sb][:, k, c0 + SUB : c0 + DSUB],
                            start=False, stop=(k == KC - 1),
                        )
                    # one fused tanh over partitions 0-111 (rows 48-63 junk);
                    # cbias = 0.5*(b_video @ W_v.T) rides the ACT bias
                    nc.scalar.activation(
                        out=th_t[sb][0:112, p * SUB : (p + 1) * SUB],
                        in_=ct[0:112, :], func=AF.Tanh,
                        bias=cbias_sb,
                    )

                def emit_zmm(q):
                    sb, p = divmod(q, PPS)
                    # two row-tiled z matmuls (array rows 0-47 / 64-111) into
                    # the two banks of one PSUM tile, concurrent on the PE
                    zt = z_ps.tile([128, 2, SUB], f32, tag="zt", name=f"z_{q}")
                    nc.tensor.matmul(
                        zt[:, 0, :], whB_sb[0:MSIZE, :],
                        th_t[sb][0:MSIZE, p * SUB : (p + 1) * SUB],
                        start=True, stop=True,
                    )
                    nc.tensor.matmul(
                        zt[:, 1, :], whB_sb[64 : 64 + MSIZE, :],
                        th_t[sb][64 : 64 + MSIZE, p * SUB : (p + 1) * SUB],
                        start=True, stop=True,
                    )
                    return zt

                def emit_expmul(q, zt, j=None):
                    # exp + in-place weighted multiply; j=None does the full
                    # DSUB in one fused exp + one mul, j=0/1 does one half
                    sb, p = divmod(q, PPS)
                    c0 = p * DSUB
                    if j is None:
                        nc.scalar.activation(
                            out=ez_t[sb][:, c0 : c0 + DSUB].rearrange(
                                "p (a b) -> p a b", a=2
                            ),
                            in_=zt, func=AF.Exp,
                        )
                        nc.vector.tensor_mul(
                            vt_t[sb][:, :, c0 : c0 + DSUB],
                            vt_t[sb][:, :, c0 : c0 + DSUB],
                            ez_t[sb][:, c0 : c0 + DSUB]
                            .unsqueeze(1)
                            .broadcast_to([128, KC, DSUB]),
                        )
                    else:
                        cj = c0 + j * SUB
                        nc.scalar.activation(
                            out=ez_t[sb][:, cj : cj + SUB], in_=zt[:, j, :],
                            func=AF.Exp,
                        )
                        nc.vector.tensor_mul(
                            vt_t[sb][:, :, cj : cj + SUB],
                            vt_t[sb][:, :, cj : cj + SUB],
                            ez_t[sb][:, cj : cj + SUB]
                            .unsqueeze(1)
                            .broadcast_to([128, KC, SUB]),
                        )

                def emit_score(q):
                    emit_expmul(q, emit_zmm(q))

                def emit_finalize(sb, g0f, ng, dma=None, ring=None):
                    # reduce cols [g0f*48, (g0f+ng)*48) of superblock sb
                    # (units of 48-col sample groups, local to sb).
                    # dma=(gd0, ngd) flushes that global group range of
                    # cT_acc to DRAM.
                    ch = g0f * MSIZE
                    g0 = sb * GPS + g0f
                    lp = nc.allow_low_precision(
                        reason="fp16 group sums; fp32 internal accum"
                    )
                    lp.__enter__()
                    tree = trp.tile([128, KC, FGRP, MSIZE // 2], f16, tag="tree",
                                    name=f"tree_{sb}_{g0f}")
                    wv = vt_t[sb][:, :, ch : ch + ng * MSIZE].rearrange(
                        "p c (g n) -> p c g n", n=MSIZE
                    )
                    tr = tree[:, :, :ng, :]
                    nc.vector.tensor_add(
                        tr, wv[:, :, :, : MSIZE // 2], wv[:, :, :, MSIZE // 2 :]
                    )
                    nc.vector.tensor_add(
                        tr[:, :, :, : MSIZE // 4],
                        tr[:, :, :, : MSIZE // 4],
                        tr[:, :, :, MSIZE // 4 :],
                    )
                    nc.vector.tensor_add(
                        tr[:, :, :, : MSIZE // 8],
                        tr[:, :, :, : MSIZE // 8],
                        tr[:, :, :, MSIZE // 8 : MSIZE // 4],
                    )
                    nc.vector.reduce_sum(
                        out=cT_acc[:, :, g0 : g0 + ng],
                        in_=tr[:, :, :, : MSIZE // 8],
                        axis=AX.X,
                    )
                    lp.__exit__(None, None, None)
                    if dma is not None:
                        gd0, ngd = dma
                        (ring or nc.gpsimd).dma_start(
                            out=cT_d[:, gd0 : gd0 + ngd].rearrange(
                                "(c p) n -> p c n", p=128
                            ),
                            in_=cT_acc[:, :, gd0 : gd0 + ngd],
                        )

                def emit_ezrow(sb, ring=None):
                    (ring or nc.gpsimd).dma_start(
                        out=ezrow_d[:, sb * SUPER : (sb + 1) * SUPER],
                        in_=ez_t[sb][0:1, :],
                    )

                # software-pipelined emission with a ONE-pair lag, interleaved
                # at half-pair granularity so no consumer head-of-line-blocks
                # the PE queue (see v1 docstring for the full rationale).
                lsb = NSB - 1

                def emit_fins(qq):
                    sb2, p2 = divmod(qq, PPS)
                    if p2 == 0:
                        emit_finalize(sb2, 0, FGRP)
                    elif p2 == 1:
                        emit_finalize(sb2, FGRP, FGRP)
                        emit_finalize(sb2, 2 * FGRP, 10)
                    else:
                        emit_finalize(sb2, 42, 11)
                        emit_finalize(sb2, 53, 11, dma=(sb2 * GPS, GPS))
                        emit_ezrow(sb2)

                def emit_consume(qq):
                    sb2, p2 = divmod(qq, PPS)
                    if sb2 < lsb:
                        emit_score(qq)
                        emit_fins(qq)
                        return
                    # last superblock: exp/mul per 512-col half, with the
                    # largest group-aligned chunk each half unlocks
                    zt = emit_zmm(qq)
                    emit_expmul(qq, zt, j=0)
                    if p2 == 0:
                        emit_finalize(lsb, 0, 10)
                    elif p2 == 1:
                        emit_finalize(lsb, 21, 11)
                    else:
                        emit_finalize(lsb, 42, 11, dma=(lsb * GPS + 42, 11))
                    emit_expmul(qq, zt, j=1)
                    if p2 == 0:
                        emit_finalize(lsb, 10, 11)
                    elif p2 == 1:
                        emit_finalize(lsb, 32, 10, dma=(lsb * GPS, 42))
                    else:
                        # ezrow first (only needs the exps, long done); the
                        # final finalize is split so the very last cT DMA is
                        # small and issues ~1us earlier.  The three tail DMAs
                        # issue on three different rings (the ~0.9us hwdge
                        # descriptor-gen cost overlaps instead of serializing)
                        # one merged finalize (fewer DVE ops + one DMA issue)
                        emit_ezrow(lsb, ring=nc.scalar)
                        emit_finalize(lsb, 53, 11, dma=(lsb * GPS + 53, 11),
                                      ring=nc.sync)

                emit_mains(0, (0, 1))
                emit_mains(0, (2, 3))
                emit_audio_a()
                for q in range(1, NPAIR):
                    emit_mains(q, (0, 1))
                    if q == 1:
                        emit_audio_inter()
                    emit_content(q - 1)
                    # consume between m2 and m3: content(q-1)'s tanh drains
                    # during m2, so the z matmuls slot in without stalling
                    # and the DVE work starts ~1.7us earlier
                    emit_mains(q, (2,))
                    emit_consume(q - 1)
                    emit_mains(q, (3,))
                emit_content(NPAIR - 1)
                emit_consume(NPAIR - 1)

    nc.compile()
    return nc


def _prep_in_maps(inputs):
    import ml_dtypes

    E4M3 = ml_dtypes.float8_e4m3

    audio = np.ascontiguousarray(np.asarray(inputs["audio"], np.float32))
    video = np.ascontiguousarray(np.asarray(inputs["video"], np.float32))

    def dev_chunks(w):  # [C*128, X] -> [128, C, X] (partition-major chunks)
        a = np.asarray(w)
        return np.ascontiguousarray(a.reshape(-1, 128, a.shape[-1]).transpose(1, 0, 2))

    W_video = np.asarray(inputs["W_video"], np.float32)   # [h, v]
    W_videoT = W_video.T                                  # [v, h]
    # fp8 DoubleRow half: rows 0:K8, pre-scaled by W8_SCALE
    WvideoT8 = np.ascontiguousarray(
        (W_videoT[:K8] * W8_SCALE).astype(E4M3)
        .reshape(K8 // 128, 128, HSIZE).transpose(1, 0, 2)
    )
    # fp16 half: rows K8:, pre-scaled by PRE_SCALE (power of 2, exact)
    W16h = np.ascontiguousarray(
        (W_videoT[K8:] * PRE_SCALE).astype(np.float16)
        .reshape(-1, 128, HSIZE).transpose(1, 0, 2)
    )
    WaudioT = np.ascontiguousarray(np.asarray(inputs["W_audio"], np.float32).T.astype(np.float16))
    WgT = dev_chunks(np.asarray(inputs["W_g"], np.float32).T.astype(np.float16))
    W_v = np.asarray(inputs["W_v"], np.float32)
    WvT = dev_chunks((0.5 * W_v.T).astype(np.float16))
    # Wcomb = 0.5 * W_video.T @ W_v.T, computed exactly in float64 on host
    Wcomb = dev_chunks(
        (0.5 * (W_videoT.astype(np.float64) @ W_v.T.astype(np.float64)))
        .astype(np.float16)
    )
    wh = np.asarray(inputs["W_h"], np.float32).T  # [48, 1]
    WhT = np.zeros((112, 1), np.float32)
    WhT[0:MSIZE] = wh
    WhT[64 : 64 + MSIZE] = wh
    WhT = np.ascontiguousarray(WhT)
    b_video = np.asarray(inputs["b_video"], np.float32)
    b_videoT = np.ascontiguousarray(b_video.reshape(-1, 128).T)
    b_audio = np.ascontiguousarray(
        np.asarray(inputs["b_audio"], np.float32).reshape(-1, 128).T
    )
    # cbias = 0.5 * (b_video @ W_v.T), on partitions 0-47 and 64-111
    cb = 0.5 * (b_video @ W_v.T)
    cbias = np.zeros((112, 1), np.float32)
    cbias[0:MSIZE, 0] = cb
    cbias[64 : 64 + MSIZE, 0] = cb
    cbias = np.ascontiguousarray(cbias)

    a2 = audio.reshape(BT, ASIZE).astype(np.float16)
    v2f = video.reshape(BT, MSIZE, VSIZE)
    v2h = v2f.astype(np.float16)
    in_maps = []
    for c in range(NCORES):
        sl = slice(c * PER, (c + 1) * PER)
        vT = np.ascontiguousarray(v2h[sl].reshape(R, VSIZE).T)
        # fp8 copy of contraction rows 0:K8, scaled by V8_SCALE, quantized
        # from the fp32 source (matches the accuracy sim)
        vT8 = np.ascontiguousarray(
            (v2f[sl].reshape(R, VSIZE).T[:K8] * V8_SCALE).astype(E4M3)
        )
        audioT = np.ascontiguousarray(a2[sl].T)
        in_maps.append(
            {
                "vT": vT,
                "vT8": vT8,
                "audioT": audioT,
                "WvideoT8": WvideoT8,
                "W16h": W16h,
                "WaudioT": WaudioT,
                "WgT": WgT,
                "WvT": WvT,
                "Wcomb": Wcomb,
                "WhT": WhT,
                "b_video": b_videoT,
                "b_audio": b_audio,
                "cbias": cbias,
            }
        )
    return in_maps


def _run(inputs, trace=False, **spmd_kwargs):
    from concourse.bass_utils import run_bass_kernel_spmd

    if "nc" not in _cached:
        _cached["nc"] = _build_nc()
    nc = _cached["nc"]
    in_maps = _prep_in_maps(inputs)
    res = run_bass_kernel_spmd(
        nc, in_maps, core_ids=list(range(NCORES)), trace=trace, **spmd_kwargs
    )
    def _part(r):
        denom = r["ezrow"].astype(np.float32).reshape(PER, MSIZE).sum(axis=1)
        return (r["cT"].astype(np.float32) / denom[None, :]).T

    parts = [_part(r) for r in res.results]
    out = np.concatenate(parts, axis=0).reshape(B, T, VSIZE)
    return np.ascontiguousarray(out.astype(np.float32)), res


def kernel(**inputs):
    out, _ = _run(inputs, trace=False)
    return out
